# revision 1
# baseline (speedup 1.0000x reference)
"""KNN loss kernel for Trainium2 (8 NeuronCores, Bass/Tile).

loss = mean_i [ mean_k(d_i,nn1, d_i,nn2) + log(sum_{j!=i} exp(-d_ij)) ]
with d_ij = ||x_i - x_j||_2, x: [8192, 64] f32.

Strategy: shard rows across 8 cores (1024 rows each). Each core computes its
[1024, 8192] block of squared distances with a single augmented matmul in
fp16 (sq_ij = |x_i|^2 + |x_j|^2 - 2<x_i,x_j> via 68-row contraction; |x|^2
terms are split hi+lo fp16 for accuracy), masks its own diagonal by
accumulating BIG*I through an extra matmul, then:
  phase A: ACT Sqrt (PSUM -> fp16 SBUF) for all 8 row tiles,
  phase B: ACT Exp(scale=-1) with accum_out (row-sum = denom) + DVE max
           (top-8 per row -> the 2 nearest-neighbor exp-values),
  tiny Ln + arithmetic -> per-partition partial sums of per_point.
The A/B phase split batches activation-table usage (Sqrt set then Exp set)
to avoid per-row-tile ACT table reloads (~2.7us each).
Host sums the 8x[128] partials and divides by N.

Columns of rhs are rolled per-core so each core's diagonal block sits at
columns [0, 1024) regardless of core id (row-sum and top-k are invariant to
column permutation).
"""

import sys

if "/opt/trn_rl_repo" not in sys.path:
    sys.path.insert(0, "/opt/trn_rl_repo")

import numpy as np

import concourse.bass as bass
import concourse.mybir as mybir
import concourse.tile as tile
from concourse import bacc
from concourse.bass_utils import run_bass_kernel_spmd

N = 8192
D = 64
NCORES = 8
RPC = N // NCORES          # rows per core (1024)
KAUG = D + 4               # augmented contraction dim (68)
NRT = RPC // 128           # row tiles per core (8)
CHUNK = 2048               # psum chunk (4 banks)
NCK = N // CHUNK           # chunks per row (4)
MMW = 512                  # matmul free width (1 psum bank fp32)
NMM = CHUNK // MMW         # matmuls per chunk (4)
BIGQ = 1000.0              # sqrt of diagonal mask added to sq

F32 = mybir.dt.float32
F16 = mybir.dt.float16

_CACHE = {}

# Set by the last kernel() call; test.py reads .exec_time_ns for profiling.
LAST_RESULTS = None


def _build_bass():
    nc = bacc.Bacc(None, target_bir_lowering=False, debug=True)
    lhsT_d = nc.declare_dram_parameter("lhsT", [KAUG, RPC], F16, isOutput=False)
    rhs_d = nc.declare_dram_parameter("rhs", [KAUG, N], F16, isOutput=False)
    eyeq_d = nc.declare_dram_parameter("eyeq", [128, 128], F16, isOutput=False)
    out_d = nc.declare_dram_parameter("out", [128, 1], F32, isOutput=True)

    AF = mybir.ActivationFunctionType

    GRP = 2                    # row-tile groups: sqrt-batch then exp-batch
    GRT = NRT // GRP           # row tiles per group (4)

    with tile.TileContext(nc) as tc:
        with (
            tc.tile_pool(name="const", bufs=1) as constp,
            tc.tile_pool(name="dist", bufs=1) as distp,
            tc.tile_pool(name="val", bufs=4) as valp,
            tc.tile_pool(name="small", bufs=1) as smallp,
            tc.tile_pool(name="psum", bufs=2, space=bass.MemorySpace.PSUM) as psump,
        ):
            rhs_sb = constp.tile([KAUG, N], F16)
            lhsT_sb = constp.tile([KAUG, RPC], F16)
            eyeq_sb = constp.tile([128, 128], F16)
            # operand order: lhsT + eyeq first (first matmul needs them),
            # then rhs in fine chunks so compute starts ASAP
            nc.sync.dma_start(lhsT_sb[:], lhsT_d[:])
            nc.sync.dma_start(eyeq_sb[:], eyeq_d[:])
            DMACH = 1024
            for ck in range(N // DMACH):
                cs = slice(ck * DMACH, (ck + 1) * DMACH)
                nc.sync.dma_start(rhs_sb[:, cs], rhs_d[:, cs])

            T8 = smallp.tile([128, 8 * NRT], F16)     # top-8 vals per row tile
            DEN = smallp.tile([128, NRT], F32)        # denominator per row tile

            for grp in range(GRP):
                # ---- phase A: matmuls + sqrt (Sqrt table set) ----
                dall = distp.tile([128, GRT * N], F16)
                for gi in range(GRT):
                    rt = grp * GRT + gi
                    for ck in range(NCK):
                        ps = psump.tile([128, CHUNK], F32)
                        for mm in range(NMM):
                            c0 = ck * CHUNK + mm * MMW
                            nc.tensor.matmul(
                                ps[:, mm * MMW:(mm + 1) * MMW],
                                lhsT_sb[:, rt * 128:(rt + 1) * 128],
                                rhs_sb[:, c0:c0 + MMW],
                                start=True,
                                stop=True,
                            )
                        if ck == 0:
                            # own diag block: add BIGQ^2*I at cols rt*128..+128
                            off = rt * 128
                            nc.tensor.matmul(
                                ps[:, off:off + 128],
                                eyeq_sb[:],
                                eyeq_sb[:],
                                start=False,
                                stop=True,
                                skip_group_check=True,
                            )
                        nc.scalar.activation(
                            dall[:, gi * N + ck * CHUNK: gi * N + (ck + 1) * CHUNK],
                            ps[:],
                            AF.Sqrt,
                        )

                # keep each group's exp ACT ops after its sqrt ACT ops so the
                # Sqrt/Exp table sets load once per group, not per row tile
                tc.no_sync_barrier()

                # ---- phase B: exp + row-sum + top-8 (Exp table set) ----
                for gi in range(GRT):
                    rt = grp * GRT + gi
                    val = valp.tile([128, N], F16)
                    nc.scalar.activation(
                        val[:], dall[:, gi * N:(gi + 1) * N], AF.Exp, scale=-1.0,
                        accum_out=DEN[:, rt:rt + 1],
                    )
                    nc.vector.max(T8[:, rt * 8:(rt + 1) * 8], val[:])
                tc.no_sync_barrier()

            # per_point partial sums: pp = -(ln v1 + ln v2)/2 + ln(denom)
            L8 = smallp.tile([128, 8 * NRT], F32)
            LD = smallp.tile([128, NRT], F32)
            nc.scalar.activation(L8[:], T8[:], AF.Ln)
            nc.scalar.activation(LD[:], DEN[:], AF.Ln)
            l8v = L8.rearrange("p (r c) -> p r c", c=8)
            s12 = smallp.tile([128, NRT], F32)
            nc.vector.tensor_add(s12[:], l8v[:, :, 0], l8v[:, :, 1])
            pp = smallp.tile([128, NRT], F32)
            # pp = (s12 * -0.5) + LD
            nc.vector.scalar_tensor_tensor(
                out=pp[:], in0=s12[:], scalar=-0.5, in1=LD[:],
                op0=mybir.AluOpType.mult, op1=mybir.AluOpType.add,
            )
            outsb = smallp.tile([128, 1], F32)
            nc.vector.reduce_sum(outsb[:], pp[:], axis=mybir.AxisListType.X)
            nc.sync.dma_start(out_d[:], outsb[:])

    nc.compile()
    return nc


def _prep_inputs(x: np.ndarray):
    x = np.ascontiguousarray(np.asarray(x, dtype=np.float32))
    assert x.shape == (N, D), x.shape
    x64 = x.astype(np.float64)
    sqn = (x64 * x64).sum(axis=1)
    sqn_hi = sqn.astype(np.float16)
    sqn_lo = (sqn - sqn_hi.astype(np.float64)).astype(np.float16)

    rhs_full = np.empty((KAUG, N), dtype=np.float16)
    rhs_full[:D] = (-2.0 * x64.T).astype(np.float16)
    rhs_full[D] = 1.0
    rhs_full[D + 1] = 1.0
    rhs_full[D + 2] = sqn_hi
    rhs_full[D + 3] = sqn_lo

    eyeq = (np.eye(128) * BIGQ).astype(np.float16)

    in_maps = []
    for d in range(NCORES):
        r0 = d * RPC
        lhsT = np.empty((KAUG, RPC), dtype=np.float16)
        lhsT[:D] = x[r0:r0 + RPC].T.astype(np.float16)
        lhsT[D] = sqn_hi[r0:r0 + RPC]
        lhsT[D + 1] = sqn_lo[r0:r0 + RPC]
        lhsT[D + 2] = 1.0
        lhsT[D + 3] = 1.0
        # roll columns so this core's diagonal block is at cols [0, RPC)
        rhs = np.ascontiguousarray(
            np.concatenate([rhs_full[:, r0:], rhs_full[:, :r0]], axis=1)
        )
        in_maps.append({"lhsT": lhsT, "rhs": rhs, "eyeq": eyeq})
    return in_maps


def kernel(x: np.ndarray) -> np.ndarray:
    global LAST_RESULTS
    if "nc" not in _CACHE:
        _CACHE["nc"] = _build_bass()
    nc = _CACHE["nc"]
    in_maps = _prep_inputs(x)
    res = run_bass_kernel_spmd(nc, in_maps, list(range(NCORES)))
    LAST_RESULTS = res
    total = 0.0
    for r in res.results:
        total += np.asarray(r["out"], dtype=np.float64).sum()
    loss = total / N
    return np.asarray(loss, dtype=np.float32)


if __name__ == "__main__":
    x = np.random.RandomState(0).randn(N, D).astype(np.float32)
    print(kernel(x))

